# revision 18
# baseline (speedup 1.0000x reference)
"""Causal self-attention (B=4, T=2048, C=1024, H=16) on 8 NeuronCores.

Sharding: core = (batch b, head-group g): data-parallel over B=4, tensor-
parallel over heads (2 groups x 8 heads).  Each core computes QKV + attention
for its 8 heads and the matching half of the c_proj contraction; the host
sums the two partial c_proj outputs per batch and adds b_proj.

Device layout notes:
  - all matmul operands bf16 (PE runs fp32 at 1/4 rate), PSUM f32
  - x, weights are pre-transposed on the host so every matmul contraction
    sits on the partition dim; no on-device transposes anywhere
  - QKV biases enter as K=1 rank-1 matmuls against a ones row
  - S is computed transposed ([keys, queries]); exp(S/8) on ScalarE with no
    max-subtraction (logits bounded ~+-4 for this problem's scale)
  - causality at tile granularity: k-tiles above the diagonal are skipped,
    diagonal tiles multiplied by precomputed 0/1 masks after exp
  - softmax denominator = ones column appended to each head's V; PV matmul
    emits [y.T | denom] per (head, q-chunk); normalization = reciprocal +
    rank-1 broadcast matmul + elementwise multiply
"""

import os

import numpy as np
import ml_dtypes

B, T, C, H = 4, 2048, 1024, 16
D = 64          # head dim
HL = 8          # heads per core
CL = HL * D     # 512 local channels
TQ = 512        # query chunk (matmul moving dim)
TK = 128        # key tile (psum partition dim)
NQC = T // TQ   # 4 query chunks
NKT = T // TK   # 16 key tiles
VW = HL * (D + 1)  # 520: V with per-head ones column

_prog = None
last_results = None  # BassKernelResults of the most recent run (for test.py)


def _build_program():
    import concourse.mybir as mybir
    import concourse.tile as tile
    from concourse import bacc

    f32 = mybir.dt.float32
    bf16 = mybir.dt.bfloat16
    EXP = mybir.ActivationFunctionType.Exp
    LN = mybir.ActivationFunctionType.Ln

    nc = bacc.Bacc("TRN2", target_bir_lowering=False, debug=False)

    xt_d = nc.dram_tensor("xt", [8, 128, T], bf16, kind="ExternalInput")
    wqk_d = nc.dram_tensor("wqk", [8, 128, 2 * CL], bf16, kind="ExternalInput")
    wv_d = nc.dram_tensor("wv", [8, 128, VW], bf16, kind="ExternalInput")
    bqk_d = nc.dram_tensor("bqk", [1, 2 * CL], bf16, kind="ExternalInput")
    bv_d = nc.dram_tensor("bv", [1, VW], bf16, kind="ExternalInput")
    wp_d = nc.dram_tensor("wp", [4, 128, C], bf16, kind="ExternalInput")
    mask_d = nc.dram_tensor("mask", [4, 128, TQ], bf16, kind="ExternalInput")
    out_d = nc.dram_tensor("out", [T, C], f32, kind="ExternalOutput")

    with tile.TileContext(nc) as tc:
        with (
            tc.tile_pool(name="persist", bufs=1) as pp,
            tc.tile_pool(name="ptpool", bufs=4) as ptp,
            tc.tile_pool(name="stage", bufs=3) as sp,
            tc.tile_pool(name="small", bufs=3) as smp,
            tc.tile_pool(name="psA", bufs=3, space="PSUM") as psA,
            tc.tile_pool(name="psY", bufs=2, space="PSUM") as psY,
            tc.tile_pool(name="psB", bufs=2, space="PSUM") as psB,
        ):
            # ---- load everything ----
            xt = [pp.tile([128, T], bf16, name=f"xt{k}") for k in range(8)]
            wqk = [pp.tile([128, 2 * CL], bf16, name=f"wqk{k}") for k in range(8)]
            wv = [pp.tile([128, VW], bf16, name=f"wv{k}") for k in range(8)]
            wp = [pp.tile([128, C], bf16, name=f"wp{k}") for k in range(4)]
            maskt = [pp.tile([128, TQ], bf16, name=f"mask{j}") for j in range(4)]
            bqk_row = pp.tile([1, 2 * CL], bf16, name="bqk_row")
            bv_row = pp.tile([1, VW], bf16, name="bv_row")
            ones_row = pp.tile([1, TQ], bf16, name="ones_row")
            # one-hot selector matrices: bcast of row 32*s of a [97,512] tile
            # into 64 partitions via a K=97 rank-1 matmul (SBUF APs may only
            # start at partition 0/32/64/96, hence the 32-pitch)
            sel = [pp.tile([97, 64], f32, name=f"sel{i}") for i in range(4)]

            for k in range(8):
                nc.sync.dma_start(out=wqk[k][:], in_=wqk_d[k])
            for k in range(8):
                nc.sync.dma_start(out=xt[k][:], in_=xt_d[k])
            for k in range(8):
                nc.sync.dma_start(out=wv[k][:], in_=wv_d[k])
            for k in range(4):
                nc.sync.dma_start(out=wp[k][:], in_=wp_d[k])
            for j in range(4):
                nc.sync.dma_start(out=maskt[j][:], in_=mask_d[j])
            nc.sync.dma_start(out=bqk_row[:], in_=bqk_d[:])
            nc.sync.dma_start(out=bv_row[:], in_=bv_d[:])
            nc.vector.memset(ones_row[:], 1.0)
            for i in range(4):
                nc.vector.memset(sel[i][:], 0.0)
                nc.vector.memset(sel[i][32 * i : 32 * i + 1, :], 1.0)

            # ---- QKV projection ----
            # QT/KT in [channel, t] layout (4 tiles of 128 channels each;
            # head h lives in tile h//2 partitions 64*(h%2) .. +64)
            qt = [pp.tile([128, T], bf16, name=f"qt{i}") for i in range(4)]
            kt = [pp.tile([128, T], bf16, name=f"kt{i}") for i in range(4)]
            # V in natural [t, channel] layout with a ones column per head
            vsb = [pp.tile([128, VW], bf16, name=f"v{i}") for i in range(NKT)]

            for i in range(4):  # channel tile
                for dst, off in ((qt, 0), (kt, CL)):
                    for j in range(NQC):  # t chunk
                        ps = psA.tile([128, TQ], f32, name="ps_qkv", tag="mm512")
                        for k in range(8):
                            nc.tensor.matmul(
                                ps[:],
                                lhsT=wqk[k][:, off + i * 128 : off + (i + 1) * 128],
                                rhs=xt[k][:, j * TQ : (j + 1) * TQ],
                                start=(k == 0),
                                stop=False,
                            )
                        nc.tensor.matmul(
                            ps[:],
                            lhsT=bqk_row[0:1, off + i * 128 : off + (i + 1) * 128],
                            rhs=ones_row[0:1, :],
                            start=False,
                            stop=True,
                        )
                        nc.vector.tensor_copy(
                            out=dst[i][:, j * TQ : (j + 1) * TQ], in_=ps[:]
                        )

            for it in range(NKT):  # t tile
                for h2 in range(2):  # half of the 520 V columns
                    w0 = h2 * (VW // 2)
                    ps = psA.tile([128, TQ], f32, name="ps_v", tag="mm512")
                    for k in range(8):
                        nc.tensor.matmul(
                            ps[:, : VW // 2],
                            lhsT=xt[k][:, it * 128 : (it + 1) * 128],
                            rhs=wv[k][:, w0 : w0 + VW // 2],
                            start=(k == 0),
                            stop=False,
                        )
                    nc.tensor.matmul(
                        ps[:, : VW // 2],
                        lhsT=ones_row[0:1, 0:128],
                        rhs=bv_row[0:1, w0 : w0 + VW // 2],
                        start=False,
                        stop=True,
                    )
                    nc.vector.tensor_copy(
                        out=vsb[it][:, w0 : w0 + VW // 2], in_=ps[:, : VW // 2]
                    )

            # ---- attention ----
            # y.T in [channel, t] layout, same packing as qt/kt
            yt = [pp.tile([128, T], bf16, name=f"yt{i}") for i in range(4)]

            for h in range(HL):  # norm deferred per head
                den_g = smp.tile([97, TQ], f32, name="den_g", tag="deng")
                nc.vector.memset(den_g[:], 1.0)  # garbage rows: ln would NaN
                slots = []
                if True:
                    g2, po = h // 2, 64 * (h % 2)
                    for qc in range(NQC):
                        ktop = (qc + 1) * (TQ // TK)  # causal: k tiles 0..ktop-1
                        yps = psY.tile([D + 1, TQ], f32, name="yps", tag="y")
                        for ktl in range(ktop):
                            j = ktl - qc * (TQ // TK)
                            # diagonal tiles have no valid columns left of
                            # col0; restrict every op to [col0, TQ)
                            col0 = j * TK if j >= 0 else 0
                            q0 = qc * TQ + col0
                            ps_s = psA.tile([128, TQ], f32, name="ps_s", tag="mm512")
                            nc.tensor.matmul(
                                ps_s[:, col0:],
                                lhsT=kt[g2][po : po + 64, ktl * TK : (ktl + 1) * TK],
                                rhs=qt[g2][po : po + 64, q0 : (qc + 1) * TQ],
                                start=True,
                                stop=True,
                            )
                            pt_t = ptp.tile([128, TQ], bf16, name="pt")
                            # P.T = exp(S.T/sqrt(D)); logits bounded, no max pass
                            nc.scalar.activation(
                                pt_t[:, col0:], ps_s[:, col0:], EXP, scale=0.125
                            )
                            if j >= 0:  # diagonal: zero the acausal corner
                                nc.vector.tensor_mul(
                                    pt_t[:, col0:], pt_t[:, col0:], maskt[j][:, col0:]
                                )
                            nc.tensor.matmul(
                                yps[:, col0:],
                                lhsT=vsb[ktl][:, h * 65 : (h + 1) * 65],
                                rhs=pt_t[:, col0:],
                                start=(ktl == 0),
                                stop=(ktl == ktop - 1),
                            )
                        # rows 0..63 = unnormalized y.T, row 64 = denominator
                        ysb = smp.tile([D + 1, TQ], f32, name="ysb", tag="ysb",
                                       bufs=10)
                        nc.vector.tensor_copy(out=ysb[:], in_=yps[:])
                        nc.vector.tensor_copy(
                            out=den_g[32 * qc : 32 * qc + 1, :], in_=ysb[64:65, :]
                        )
                        slots.append((h, qc, ysb))
                # batched 1/den for the head: exp(-ln(den)) on ScalarE
                ln_g = smp.tile([97, TQ], f32, name="ln_g", tag="lng")
                rec_g = smp.tile([97, TQ], f32, name="rec_g", tag="recg")
                nc.scalar.activation(ln_g[:], den_g[:], LN)
                nc.scalar.activation(rec_g[:], ln_g[:], EXP, scale=-1.0)
                for h_, qc, ysb in slots:
                    g2, po = h_ // 2, 64 * (h_ % 2)
                    bc = psB.tile([64, TQ], f32, name="bc", tag="b")
                    nc.tensor.matmul(
                        bc[:], lhsT=sel[qc][:], rhs=rec_g[:], start=True, stop=True
                    )
                    nc.vector.tensor_mul(
                        yt[g2][po : po + 64, qc * TQ : (qc + 1) * TQ],
                        ysb[0:64, :],
                        bc[:],
                    )

            # ---- c_proj (local half of the contraction) ----
            for it in range(NKT):
                for oc in range(2):
                    pso = psA.tile([128, TQ], f32, name="ps_o", tag="mm512")
                    for ic in range(4):
                        nc.tensor.matmul(
                            pso[:],
                            lhsT=yt[ic][:, it * 128 : (it + 1) * 128],
                            rhs=wp[ic][:, oc * TQ : (oc + 1) * TQ],
                            start=(ic == 0),
                            stop=(ic == 3),
                        )
                    ot = sp.tile([128, TQ], f32, name="ot")
                    nc.vector.tensor_copy(out=ot[:], in_=pso[:])
                    nc.sync.dma_start(
                        out=out_d[it * 128 : (it + 1) * 128, oc * TQ : (oc + 1) * TQ],
                        in_=ot[:],
                    )

    nc.finalize()
    return nc


def _bf16(a):
    return np.ascontiguousarray(a, dtype=np.float32).astype(ml_dtypes.bfloat16)


def _core_inputs(x, w_attn, b_attn, w_proj, masks, core):
    b, g = divmod(core, 2)
    gs = slice(g * CL, (g + 1) * CL)
    wq, wk, wv_ = (w_attn[i * C : (i + 1) * C][gs] for i in range(3))
    bq, bk, bv_ = (b_attn[i * C : (i + 1) * C][gs] for i in range(3))

    wqkT = np.concatenate([wq, wk], 0).T            # [C, 2*CL]
    wvT = wv_.T                                     # [C, CL]
    wv_aug = np.zeros((C, VW), np.float32)
    bv_aug = np.zeros((1, VW), np.float32)
    for h in range(HL):
        wv_aug[:, h * 65 : h * 65 + 64] = wvT[:, h * 64 : (h + 1) * 64]
        bv_aug[0, h * 65 : h * 65 + 64] = bv_[h * 64 : (h + 1) * 64]
        bv_aug[0, h * 65 + 64] = 1.0                # softmax denominator column

    return {
        "xt": _bf16(x[b].T).reshape(8, 128, T),
        "wqk": _bf16(wqkT).reshape(8, 128, 2 * CL),
        "wv": _bf16(wv_aug).reshape(8, 128, VW),
        "bqk": _bf16(np.concatenate([bq, bk])[None, :]),
        "bv": _bf16(bv_aug),
        "wp": _bf16(w_proj[:, gs].T).reshape(4, 128, C),
        "mask": masks,
    }


def _make_masks():
    qq = np.arange(TQ)[None, :]
    kk = np.arange(TK)[:, None]
    m = np.stack([(qq >= kk + j * TK) for j in range(4)]).astype(np.float32)
    return m.astype(ml_dtypes.bfloat16)


def kernel(x, w_attn, b_attn, w_proj, b_proj):
    global _prog, last_results
    from concourse.bass_utils import run_bass_kernel_spmd

    if _prog is None:
        _prog = _build_program()

    x = np.asarray(x, np.float32)
    w_attn = np.asarray(w_attn, np.float32)
    b_attn = np.asarray(b_attn, np.float32)
    w_proj = np.asarray(w_proj, np.float32)
    b_proj = np.asarray(b_proj, np.float32)

    masks = _make_masks()
    in_maps = [
        _core_inputs(x, w_attn, b_attn, w_proj, masks, core) for core in range(8)
    ]
    kwargs = {}
    tmpdir = os.environ.get("BASS_TMPDIR")
    if tmpdir:
        os.makedirs(tmpdir, exist_ok=True)
        kwargs["tmpdir"] = tmpdir
    res = run_bass_kernel_spmd(_prog, in_maps, list(range(8)), **kwargs)
    last_results = res

    out = np.empty((B, T, C), np.float32)
    for b in range(B):
        out[b] = res.results[2 * b]["out"] + res.results[2 * b + 1]["out"] + b_proj
    return out
